# revision 45
# baseline (speedup 1.0000x reference)
"""Causal self-attention with RoPE — Trainium2 Bass/Tile kernel (v2).

Problem: B=2, T=2048, C=2048, H=16 heads, D=128 head dim.
    qkv = x @ w_qkv ; RoPE(q, k) ; causal softmax attention ; out = attn_out @ w_out

Sharding (8 cores): core c handles batch b = c//4 and the 4 heads
hg = c%4 (heads 4*hg .. 4*hg+3).  Each core computes
    partial_c = attn_bh(x[b]) @ w_out[rows of its heads]      (shape [T, C])
and the host all-reduces: out[b] = sum of the 4 partials of batch b.

v2 design (vs v1 serial phases):
  * Software-pipelined single pass over 512-token chunks: block b emits
    QKV(b) ⋈ attention(b-1) ⋈ out-proj(b-2), interleaved at matmul-group
    granularity so PE never starves while ScalarE runs exps.
  * Row-sums via DVE accumulation of exp tiles (bf16) + ONE ones-matmul
    per (head, q-chunk) — removes 2 of 6 matmuls per attention pair.
  * reciprocal_approx_fast (single custom-DVE op) for 1/rowsum.
  * RoPE fused with the PSUM->SBUF eviction of q/k (no separate cast).
  * x streamed once; bf16 output partials (halves out DMA).
"""

import sys

for _p in ("/opt/trn_rl_repo",):
    if _p not in sys.path:
        sys.path.insert(0, _p)

import numpy as np
import ml_dtypes

import concourse.bass as bass
import concourse.mybir as mybir
import concourse.tile as tile

BF = mybir.dt.bfloat16
FP = mybir.dt.float32

BF_NP = ml_dtypes.bfloat16

NUM_HEADS = 16
B, T_FULL, C_FULL = 2, 2048, 2048
D = 128
N_CORES = 8
HPC = 4  # heads per core

ROPE_THETA = 10000.0


def _split_multi_waits(nc):
    """This container's walrus supports only ONE sync-wait per instruction
    ("Too many sync wait commands").  Hoist all but one wait of every
    multi-wait instruction onto preceding EventSemaphore instructions
    executed by the same engine's sequencer (block order = program order per
    engine) — same semantics, codegen-legal."""
    import bass_rust

    skip = (mybir.InstEventSemaphore,)
    ctr = 0
    for fn in nc.m.functions:
        for blk in fn.blocks:
            new_insts = None
            for idx, inst in enumerate(blk.instructions):
                si = inst.sync_info
                if (
                    not isinstance(inst, skip)
                    and si is not None
                    and si.on_wait
                    and len(si.on_wait) > 1
                ):
                    if new_insts is None:
                        new_insts = list(blk.instructions[:idx])
                    # keep the first wait (the data-dep one, usually latest to
                    # resolve) on the instruction itself; hoist the rest.
                    for w in si.on_wait[1:]:
                        ev = mybir.InstEventSemaphore(
                            name=f"I-dmaw{ctr}", ins=[], outs=[]
                        )
                        ctr += 1
                        ev.sync_info = bass_rust.SyncInfo(
                            on_wait=[w], on_update=[]
                        )
                        ev.engine = inst.engine
                        new_insts.append(ev)
                    inst.sync_info = bass_rust.SyncInfo(
                        on_wait=[si.on_wait[0]], on_update=si.on_update or []
                    )
                    new_insts.append(inst)
                elif new_insts is not None:
                    new_insts.append(inst)
            if new_insts is not None:
                blk.instructions = new_insts


class Cfg:
    """Kernel geometry. Full-size by default; shrinkable for simulator tests."""

    def __init__(self, T=T_FULL, C=C_FULL, hpc=HPC):
        assert T % 512 == 0 and C % 512 == 0
        self.T = T
        self.C = C
        self.hpc = hpc
        self.scale = 1.0 / np.sqrt(D)
        self.c_tiles = C // 128      # contraction tiles for QKV
        self.t_chunks = T // 512     # token chunks (QKV + queries)
        self.t_tiles = T // 128      # token tiles (keys / out rows)
        self.n_chunks = C // 512     # output-feature chunks for out-proj


def build_attention(cfg: Cfg):
    """Build the SPMD Bass program (identical on all cores; data differs)."""
    nc = bass.Bass("TRN2", debug=False, enable_partition_id=False)
    T, C, hpc = cfg.T, cfg.C, cfg.hpc
    F = hpc * D  # per-core q (or k, or v) feature count

    hc = C // 2
    n_grp = cfg.c_tiles // 2  # x / wv stream in 2-cc groups (2KB DMA lines)
    # DRAM inputs pre-packed on host into few, large, contiguous-line tiles:
    # each dma_start costs ~650ns of issuing-engine time (DMA_DIRECT2D) and
    # 1KB-line transfers run the queues well below HBM rate.
    # xg[(tci*n_grp+g)*128 + p, :] = [x^T tile (tci,2g) | x^T tile (tci,2g+1)]
    xg = nc.dram_tensor(
        "xg", [cfg.t_chunks * n_grp * 128, 1024], BF, kind="ExternalInput"
    )
    # wqkt[ft*128 + p, cc*128 + f]: q/k weights, one contiguous 512KB per ft
    wqkt = nc.dram_tensor("wqkt", [2 * hpc * 128, C], BF, kind="ExternalInput")
    # wvg[g*128 + p, :] = [wv tile 2g | wv tile 2g+1]
    wvg = nc.dram_tensor("wvg", [n_grp * 128, 2 * F], BF, kind="ExternalInput")
    wout = nc.dram_tensor("wout", [F, C], BF, kind="ExternalInput")
    cosp = nc.dram_tensor("cosp", [cfg.t_chunks * 128, 512], BF, kind="ExternalInput")
    sinp = nc.dram_tensor("sinp", [cfg.t_chunks * 128, 512], BF, kind="ExternalInput")
    masks = nc.dram_tensor("masks", [128, 128], BF, kind="ExternalInput")
    out = nc.dram_tensor("out", [T, C], BF, kind="ExternalOutput")

    Exp = mybir.ActivationFunctionType.Exp
    Ln = mybir.ActivationFunctionType.Ln

    with tile.TileContext(nc) as tc:
        with (
            tc.tile_pool(name="sb", bufs=1) as sb,
            tc.tile_pool(name="ps", bufs=1, space="PSUM") as ps,
        ):
            # ---- weights + constants ----
            # Ring plan (v3): the three DMA queues (sync/scalar HWDGE,
            # gpsimd SWDGE) share the 16 SDMA engines round-robin, so each
            # busy queue gets ~1/3 of the ~335 GB/s aggregate.  Front-load
            # ONLY the block-0 critical path (wqkf0, x(0), cos/sin slice 0)
            # and stream everything else behind it in consumption order.
            wqkf_sb = [
                sb.tile([128, C], BF, name=f"wqkf_sb{ft}", tag=f"wqk{ft}")
                for ft in range(2 * hpc)
            ]
            cos_sb = sb.tile([D, T], BF, name="cos_sb")
            sin_sb = sb.tile([D, T], BF, name="sin_sb")
            masks_sb = sb.tile([128, 128], BF, name="masks_sb")
            ones_sb = sb.tile([128, 128], BF, name="ones_sb")
            wv_sb = [
                sb.tile([128, 2 * F], BF, name=f"wv_sb{g}", tag=f"wv{g}")
                for g in range(n_grp)
            ]
            wout_sb = [
                sb.tile([128, C], BF, name=f"wout_sb{h}", tag=f"wo{h}")
                for h in range(hpc)
            ]

            # PE warm-up: HAM un-throttles (1.2 -> 2.4 GHz) only after ~3.4us
            # of sustained matmul activity; real work can't start until DMA
            # delivers wqkf0+x at ~8.5us.  A short burst of dummy matmuls on
            # a memset scratch tile starts the busy window at ~6.3us so the
            # real stream runs warm almost immediately.  ones_sb is also
            # generated on-chip (memset) instead of DMA'd.
            nc.vector.memset(ones_sb, 1.0)
            warm_sb = sb.tile([128, 512], BF, name="warm_sb")
            # gpsimd's engine program reaches user code ~1.3us before DVE's,
            # so the dummy-matmul scratch memset goes there
            nc.gpsimd.memset(warm_sb, 0.0)
            wps = ps.tile([128, 512], FP, name="warmps", tag="ad", bufs=2)
            # enough dummies to bridge [first-PE ~7.3us, first-data ~12.5us]
            # AND sustain the HAM activity window so the arrival-bound first
            # pass runs at 2.4GHz instead of sputtering cold at 1.2
            n_warm = 12
            for i in range(n_warm):
                nc.tensor.matmul(
                    wps,
                    lhsT=warm_sb[:, 0:128],
                    rhs=warm_sb,
                    start=(i == 0),
                    stop=(i == n_warm - 1),
                )

            def load_wqkf_halves(ft, eng):
                # block-0 fts in two strided halves (2KB lines) so the first
                # LDWEIGHTS can fire at ~half the full-tile arrival time
                r = slice(ft * 128, (ft + 1) * 128)
                for h in range(2):
                    eng.dma_start(
                        out=wqkf_sb[ft][:, h * hc : (h + 1) * hc],
                        in_=wqkt[r, h * hc : (h + 1) * hc],
                    )

            def load_wqkf(ft, eng):
                eng.dma_start(
                    out=wqkf_sb[ft], in_=wqkt[ft * 128 : (ft + 1) * 128, :]
                )

            def load_cos_sin(tci, eng):
                r = slice(tci * 128, (tci + 1) * 128)
                sl = slice(tci * 512, (tci + 1) * 512)
                eng.dma_start(out=cos_sb[:, sl], in_=cosp[r, :])
                eng.dma_start(out=sin_sb[:, sl], in_=sinp[r, :])

            def deferred_weight_loads():
                """Emitted after the block-0 x loads; per-queue FIFO order =
                consumption order.  Per-ft deadline is ~3.4us apart; cos/sin
                slice 0 (needed at the first RoPE, which gates ft2's psum
                rotation) rides at the FRONT of gpsimd via x_loads(0);
                wqkf2 goes on sync right behind its x share."""
                if 2 * hpc > 2:
                    load_wqkf(2, nc.sync)
                for ft in range(3, 2 * hpc):
                    load_wqkf(ft, nc.gpsimd if ft % 2 == 1 else nc.scalar)
                for g in range(n_grp):
                    eng = (nc.sync, nc.gpsimd, nc.scalar)[g % 3]
                    eng.dma_start(
                        out=wv_sb[g], in_=wvg[g * 128 : (g + 1) * 128, :]
                    )
                nc.sync.dma_start(out=masks_sb, in_=masks[:, :])
                for tci in range(1, cfg.t_chunks):
                    load_cos_sin(tci, nc.gpsimd)
                for h in range(hpc):
                    eng = nc.scalar if h % 2 == 0 else nc.gpsimd
                    eng.dma_start(
                        out=wout_sb[h], in_=wout[h * 128 : (h + 1) * 128, :]
                    )

            # ---- persistent state ----
            # q/k transposed [D, T] per head (RoPE'd); v natural [T, F].
            qk_t = [
                sb.tile([D, T], BF, name=f"qk_t{ft}", tag=f"qkt{ft}")
                for ft in range(2 * hpc)
            ]
            v_sb = sb.tile([128, cfg.t_tiles, F], BF, name="v_sb")
            otn = [[None] * cfg.t_chunks for _ in range(hpc)]

            # =============== emission units ===============

            # group g holds ccs (2g, 2g+1).  Ring plans: block 0 front-loads
            # wqkf0 (scalar) and wqkf1 (sync) for the interleaved first
            # pass, so gpsimd carries the x middle; blocks 1+ split ~3/3/2.
            def ring_plan(tci):
                if tci == 0:
                    # sync: [xg, w1h0, w1h1, xg...]; scalar: [w0h0, w0h1,
                    # xg...]; gpsimd: [xg...] — positions model the per-queue
                    # FIFO backlog in 256KB units (wqk half = 1 unit)
                    n_sy0 = max(1, n_grp // 4)
                    n_sc0 = min(n_grp - n_sy0 - 1, max(0, n_grp // 4))
                    sy = list(range(0, n_sy0))
                    gp = list(range(n_sy0, n_grp - n_sc0))
                    sc = list(range(n_grp - n_sc0, n_grp))
                    pos = {}
                    for i, g in enumerate(sy):
                        pos[g] = 0.0 + i if i == 0 else 2.1 + i
                    for i, g in enumerate(gp):
                        pos[g] = 1.9 + i  # behind cos/sin(0) + SWDGE start
                    for i, g in enumerate(sc):
                        pos[g] = 2.0 + i
                else:
                    n_sy = max(1, (n_grp * 3 + 7) // 8)
                    n_gp = min(n_grp, n_sy + max(1, (n_grp * 3) // 8))
                    sy = list(range(0, n_sy))
                    gp = list(range(n_sy, n_gp))
                    sc = list(range(n_gp, n_grp))
                    pos = {}
                    for off, gl in zip((0.0, 0.3, 0.6), (sy, gp, sc)):
                        for i, g in enumerate(gl):
                            pos[g] = off + i
                return (sy, gp, sc), pos

            def x_dma(tci, g, eng):
                x_t = sb.tile([128, 1024], BF, name=f"xg{g}",
                              tag=f"xg{g}", bufs=2)
                r = (tci * n_grp + g) * 128
                eng.dma_start(out=x_t, in_=xg[r : r + 128, :])
                return x_t

            def x_loads(tci):
                """x group tiles for chunk tci spread across the three DMA
                rings per ring_plan.  Block 0: each HWDGE queue leads with an
                x group, then its wqkf0/1 halves — the interleaved first pass
                wants x earliest, ft weights by the 1st/3rd group."""
                (sy, gp, sc), _ = ring_plan(tci)
                groups = [None] * n_grp
                if tci == 0:
                    groups[sy[0]] = x_dma(0, sy[0], nc.sync)
                    load_wqkf_halves(1, nc.sync)
                    for g in sy[1:]:
                        groups[g] = x_dma(0, g, nc.sync)
                    load_wqkf_halves(0, nc.scalar)
                    for g in sc:
                        groups[g] = x_dma(0, g, nc.scalar)
                    load_cos_sin(0, nc.gpsimd)
                    for g in gp:
                        groups[g] = x_dma(0, g, nc.gpsimd)
                    return groups
                for eng, gl in ((nc.sync, sy), (nc.gpsimd, gp), (nc.scalar, sc)):
                    for g in gl:
                        groups[g] = x_dma(tci, g, eng)
                return groups

            def xap(x_ch, cc, c0, c1):
                """Slice [c0:c1) of x tile cc out of its group tile."""
                off = (cc % 2) * 512
                return x_ch[cc // 2][:, off + c0 : off + c1]

            def arrival_order(tci):
                """Expected arrival interleave of the three rings' x groups
                (unit = one 256KB transfer at 1/3 aggregate bw); groups
                expand to their cc pairs."""
                _, pos = ring_plan(tci)
                ev = sorted((p, g) for g, p in pos.items())
                return [cc for _, g in ev for cc in (2 * g, 2 * g + 1)]

            cc_order0 = arrival_order(0)
            cc_order = arrival_order(1)

            def rope_evict(psq, ft, sl):
                # RoPE fused with PSUM eviction:
                #   qk_t[d] = psq[d]*cos[d] + psq[(d+64)%128]*sin_baked[d]
                t1 = sb.tile([128, 512], BF, name="t1", tag="rt1", bufs=2)
                nc.vector.tensor_mul(t1, psq, cos_sb[:, sl])
                t2 = sb.tile([128, 512], BF, name="t2", tag="rt2", bufs=2)
                nc.vector.tensor_mul(t2[0:64, :], psq[64:128, :], sin_sb[0:64, sl])
                nc.vector.tensor_mul(t2[64:128, :], psq[0:64, :], sin_sb[64:128, sl])
                nc.vector.tensor_add(qk_t[ft][:, sl], t1, t2)

            def qk_unit(tci, ft, x_ch):
                """One q-or-k feature tile for chunk tci + fused RoPE."""
                order = cc_order0 if tci == 0 else cc_order
                sl = slice(tci * 512, (tci + 1) * 512)
                psq = ps.tile([128, 512], FP, name="psq", tag="ad", bufs=2)
                for ci, cc in enumerate(order):
                    nc.tensor.matmul(
                        psq,
                        lhsT=wqkf_sb[ft][:, cc * 128 : (cc + 1) * 128],
                        rhs=xap(x_ch, cc, 0, 512),
                        start=(ci == 0),
                        stop=(ci == cfg.c_tiles - 1),
                    )
                rope_evict(psq, ft, sl)

            def qkn_unit(tci, fts, lags, x_ch):
                """Several feature tiles with their cc-accumulations
                interleaved (ft k lagging by lags[k] ccs): block 0's first
                pass is DMA-arrival-bound (~3.5MB critical bytes), so consume
                each freshly-landed x tile several times instead of idling —
                staggered so each ft's weight tile (queued behind x on its
                ring) has time to land."""
                order = cc_order0 if tci == 0 else cc_order
                sl = slice(tci * 512, (tci + 1) * 512)
                nc_ = cfg.c_tiles
                pss = []
                for k in range(len(fts)):
                    if k < 2:
                        p = ps.tile([128, 512], FP, name=f"psq{k}",
                                    tag="ad", bufs=2)
                    else:
                        p = ps.tile([128, 1024], FP, name=f"psq{k}",
                                    tag="sc", bufs=2)[:, 0:512]
                    pss.append(p)
                for i in range(nc_ + lags[-1]):
                    for k, (ft, lag) in enumerate(zip(fts, lags)):
                        j = i - lag
                        if 0 <= j < nc_:
                            cc = order[j]
                            nc.tensor.matmul(
                                pss[k],
                                lhsT=wqkf_sb[ft][:, cc * 128 : (cc + 1) * 128],
                                rhs=xap(x_ch, cc, 0, 512),
                                start=(j == 0),
                                stop=(j == nc_ - 1),
                            )
                for k, ft in enumerate(fts):
                    rope_evict(pss[k], ft, sl)

            def v_unit(tci, tt, x_ch):
                """One 128-token v tile for chunk tci."""
                order = cc_order0 if tci == 0 else cc_order
                psv = ps.tile([128, F], FP, name="psv", tag="ad", bufs=2)
                for ci, cc in enumerate(order):
                    nc.tensor.matmul(
                        psv,
                        lhsT=xap(x_ch, cc, tt * 128, (tt + 1) * 128),
                        rhs=wv_sb[cc // 2][:, (cc % 2) * F : (cc % 2 + 1) * F],
                        start=(ci == 0),
                        stop=(ci == cfg.c_tiles - 1),
                    )
                nc.scalar.copy(v_sb[:, tci * 4 + tt, :], psv)

            osb_box = [None]
            store_ctr = [0]

            def d_unit(qc, tt4, n, alt=False, pstag="ad"):
                """Out-proj for (row tile qc*4+tt4, 512-col chunk n)."""
                tt = qc * 4 + tt4
                if n == 0:
                    osb_box[0] = sb.tile([128, C], BF, name="osb",
                                         tag="osb", bufs=2)
                osb = osb_box[0]
                if pstag == "sc":
                    # the pure-D last block cycles "ad"/"sc"/"av" psum tags:
                    # 6-deep rotation lets the PE run ahead of evictions
                    # queued on the busy tail engines
                    pso = ps.tile([128, 1024], FP, name="pso",
                                  tag="sc", bufs=2)[:, 0:512]
                else:
                    pso = ps.tile([128, 512], FP, name="pso", tag=pstag,
                                  bufs=2)
                for h in range(hpc):
                    nc.tensor.matmul(
                        pso,
                        lhsT=otn[h][qc][:, tt4 * 128 : (tt4 + 1) * 128],
                        rhs=wout_sb[h][:, n * 512 : (n + 1) * 512],
                        start=(h == 0),
                        stop=(h == hpc - 1),
                    )
                # alternate evictions between ScalarE and DVE: either alone
                # becomes the D-pipeline bottleneck when it also carries the
                # exp/epilogue stream (trace v5: DVE-only clogged the tail)
                if (tt4 + n) % 2 == 1:
                    nc.vector.tensor_copy(osb[:, n * 512 : (n + 1) * 512], pso)
                else:
                    nc.scalar.copy(osb[:, n * 512 : (n + 1) * 512], pso)
                # batched half-row stores (2KB-4KB lines), rotated across
                # idle DMA queues: per-slice stores cost ~650ns of issuing-
                # engine time each and serialized ~10us at the v2 tail.
                # Tail blocks stay on the HWDGE queues — a last-moment SWDGE
                # store adds a ~4.5us gpsimd queue-drain to the postamble.
                half = cfg.n_chunks // 2
                lastu = alt and qc == cfg.t_chunks - 1 and tt4 == 3
                if lastu and n >= half:
                    # very last row tile: per-slice stores on alternating
                    # HWDGE queues so the final drain is one 128KB receipt
                    eng = (nc.sync, nc.scalar)[n % 2]
                    eng.dma_start(
                        out=out[tt * 128 : (tt + 1) * 128,
                                n * 512 : (n + 1) * 512],
                        in_=osb[:, n * 512 : (n + 1) * 512],
                    )
                elif n == cfg.n_chunks - 1 or (half and n == half - 1):
                    c0 = 0 if n == half - 1 else half * 512
                    c1 = (n + 1) * 512
                    engs = (nc.sync, nc.scalar) if alt else (
                        nc.sync, nc.scalar, nc.gpsimd
                    )
                    eng = engs[store_ctr[0] % len(engs)]
                    store_ctr[0] += 1
                    eng.dma_start(
                        out=out[tt * 128 : (tt + 1) * 128, c0:c1],
                        in_=osb[:, c0:c1],
                    )

            def b_steps(h, qc):
                """Attention for (head h, query chunk qc): generator yielding
                once per key-tile pair so the scheduler can interpose PE work
                between the scores matmul and the exp-dependent av matmul."""
                q_sl = qk_t[h][:, qc * 512 : (qc + 1) * 512]
                k_h = qk_t[hpc + h]
                nkp = 2 * (qc + 1)
                avps = ps.tile([128, 512], FP, name="avps", tag="av", bufs=2)
                esum = sb.tile([128, 1024], BF, name="esum", tag="esum", bufs=2)
                exps = {}

                # trimmed diagonal pairs (qc>=1): diagonal key-tile m only
                # needs q >= 128*m.  Layouts are q-aligned (tile m's q-slice
                # q0: lives at column 512+q0 when in the second half) so the
                # final halves-fold still produces per-q rowsums.
                #   pair 'A' (m=0,1): [0:512] full + [640:1024] = q[128:512]
                #   pair 'B' (m=2,3): [256:512] = q[256:512] + [896:1024]
                #                      = q[384:512]
                # (stop= is sim-only metadata; start= is the HW psum reset,
                # and the first-emitted pair's m0 av is always full-width)
                trim = True
                tri = masks_sb  # [128,128] k<=q' triangle

                def _ranges(jp):
                    if trim and jp == nkp - 2:
                        return "A", ((0, 512, 0), (640, 1024, 128))
                    if trim and jp == nkp - 1:
                        return "B", ((256, 512, 256), (896, 1024, 384))
                    return None, ((0, 512, 0), (512, 1024, 0))

                def sc_exp(jp):
                    j0, j1 = 2 * jp, 2 * jp + 1
                    kind, rng = _ranges(jp)
                    scps = ps.tile([128, 1024], FP, name="scps",
                                   tag="sc", bufs=2)
                    for (c0, c1, q0), j in zip(rng, (j0, j1)):
                        nc.tensor.matmul(
                            scps[:, c0:c1],
                            lhsT=k_h[:, j * 128 : (j + 1) * 128],
                            rhs=q_sl[:, q0:512],
                            start=True,
                            stop=True,
                        )
                    expT = sb.tile([128, 1024], BF, name="expT",
                                   tag="exp", bufs=4)
                    if kind is None:
                        nc.scalar.activation(expT, scps, Exp,
                                             scale=float(cfg.scale))
                    else:
                        for c0, c1, _ in rng:
                            nc.scalar.activation(
                                expT[:, c0:c1], scps[:, c0:c1], Exp,
                                scale=float(cfg.scale),
                            )
                            # triangle mask on the leading 128 cols (the
                            # tile's own diagonal block); the rest is causal
                            nc.vector.tensor_mul(
                                expT[:, c0 : c0 + 128], expT[:, c0 : c0 + 128],
                                tri,
                            )
                    exps[jp] = expT

                def av_presum(jp, first, last):
                    expT = exps.pop(jp)
                    j0, j1 = 2 * jp, 2 * jp + 1
                    kind, rng = _ranges(jp)
                    for (c0, c1, q0), j in zip(rng, (j0, j1)):
                        nc.tensor.matmul(
                            avps[:, q0:512],
                            lhsT=v_sb[:, j, h * 128 : (h + 1) * 128],
                            rhs=expT[:, c0:c1],
                            start=first and c0 == 0,
                            stop=last and c1 == 1024,
                        )
                    if kind is None:
                        if first:
                            nc.vector.tensor_copy(esum, expT)
                        else:
                            nc.vector.tensor_add(esum, esum, expT)
                    elif kind == "A":
                        nc.vector.tensor_copy(esum[:, 0:512], expT[:, 0:512])
                        nc.vector.tensor_copy(
                            esum[:, 640:1024], expT[:, 640:1024]
                        )
                        nc.vector.memset(esum[:, 512:640], 0)
                    else:
                        nc.vector.tensor_add(
                            esum[:, 256:512], esum[:, 256:512],
                            expT[:, 256:512],
                        )
                        nc.vector.tensor_add(
                            esum[:, 896:1024], esum[:, 896:1024],
                            expT[:, 896:1024],
                        )

                # diagonal pairs first: their post-exp mask multiply (DVE)
                # adds latency before the av matmul can run; fronting them
                # hides it behind the rest of the pair pipeline.
                order = [nkp - 2, nkp - 1] + list(range(nkp - 2))
                for pos, jp in enumerate(order):
                    sc_exp(jp)
                    yield
                    if pos >= 1:
                        pjp = order[pos - 1]
                        av_presum(pjp, first=(pos == 1), last=False)
                av_presum(order[-1], first=(nkp == 1), last=True)
                # epilogue: rowsums -> 1/rowsum -> normalized attn out
                nc.vector.tensor_add(
                    esum[:, 0:512], esum[:, 0:512], esum[:, 512:1024]
                )
                dnps = ps.tile([128, 1024], FP, name="dnps", tag="sc", bufs=2)
                nc.tensor.matmul(
                    dnps[:, 0:512], lhsT=ones_sb, rhs=esum[:, 0:512],
                    start=True, stop=True,
                )
                # 1/rowsum as exp(-ln(x)) on ScalarE: ln+exp share one act
                # table (natural_log_exp_and_others) with the scores exp,
                # so no table reloads; DVE reciprocal is 6x slower.
                lnd = sb.tile([128, 512], FP, name="lnd", tag="lnd", bufs=2)
                nc.scalar.activation(lnd, dnps[:, 0:512], Ln)
                rsrec = sb.tile([128, 512], FP, name="rsrec",
                                tag="rsrec", bufs=2)
                nc.scalar.activation(rsrec, lnd, Exp, scale=-1.0)
                o = sb.tile([128, 512], BF, name=f"otn{h}_{qc}",
                            tag=f"otn{h}", bufs=4)
                nc.vector.tensor_mul(o, avps, rsrec)
                otn[h][qc] = o

            # =============== scheduler ===============

            def chain_b(qc):
                for h in range(hpc):
                    yield from b_steps(h, qc)

            def interleave_even(a, b):
                """Merge two unit lists evenly (Bresenham)."""
                if not a:
                    return list(b)
                if not b:
                    return list(a)
                res, ai, bi = [], 0, 0
                na, nb = len(a), len(b)
                while ai < na or bi < nb:
                    if bi >= nb or (ai < na and ai * nb <= bi * na):
                        res.append(a[ai]); ai += 1
                    else:
                        res.append(b[bi]); bi += 1
                return res

            def merge(units, bgen, n_bsteps):
                """Interleave unit closures with pulls from the B generator,
                spread evenly by count."""
                if bgen is None:
                    for u in units:
                        u()
                    return
                # hold back a unit or three to emit after the generator
                # drains: they keep the PE fed while the last (h,qc) epilogue
                # chain (fold -> ones -> ln -> exp -> mul) resolves on
                # Act/DVE.  Only small D units are held back aggressively;
                # big A units must stay spread through the B pairs.
                hold = 1 if b < cfg.t_chunks else min(8, len(units))
                nu = max(0, len(units) - hold)
                ui = 0
                for bi in range(n_bsteps):
                    # emit units scheduled before this b-step
                    while ui < nu and ui * n_bsteps < bi * nu:
                        units[ui](); ui += 1
                    try:
                        next(bgen)
                    except StopIteration:
                        break
                # drain
                for step in bgen:
                    pass
                while ui < len(units):
                    units[ui](); ui += 1

            n_blocks = cfg.t_chunks + 2
            for b in range(n_blocks):
                units = []
                if b < cfg.t_chunks:
                    x_ch = x_loads(b)
                    if b == 0:
                        deferred_weight_loads()
                        nf0 = min(3, 2 * hpc)
                        lgs = (0, 3, 8)[:nf0]
                        a_units = [
                            (lambda xc=x_ch, nf0=nf0, lgs=lgs:
                             qkn_unit(0, list(range(nf0)), lgs, xc))
                        ] + [
                            (lambda ft=ft, xc=x_ch: qk_unit(0, ft, xc))
                            for ft in range(nf0, 2 * hpc)
                        ]
                    else:
                        a_units = [
                            (lambda tci=b, ft=ft, xc=x_ch: qk_unit(tci, ft, xc))
                            for ft in range(2 * hpc)
                        ]
                    a_units += [
                        (lambda tci=b, tt=tt, xc=x_ch: v_unit(tci, tt, xc))
                        for tt in range(4)
                    ]
                    units = a_units
                # D(qc) is emitted half in block qc+2, half in block qc+3
                # (when it exists): the tail block B(3) is Act-bound, and
                # spare D units both feed the PE there and cover the final
                # epilogue chain in the last block.
                alt = b >= cfg.t_chunks
                last_blk = b == n_blocks - 1
                dunits = []
                for qc, part in ((b - 3, 1), (b - 2, 0)):
                    if not (0 <= qc < cfg.t_chunks):
                        continue
                    # split D(qc) across blocks qc+2/qc+3 mid-kernel, but
                    # keep the LAST block D(last)-only: a dense tail stream
                    # (v2 trace: mixed leftovers stalled on late epilogues
                    # and HAM re-throttled the PE to 1.2 GHz twice)
                    split = qc + 3 <= n_blocks - 2
                    if part == 0:
                        rng = range(0, 2 if split else 4)
                    elif split:
                        rng = range(2, 4)
                    else:
                        continue
                    dunits += [
                        (lambda qc=qc, tt4=tt4, n=n, alt=alt,
                                pstag=(("ad", "sc", "av")
                                       [(tt4 * cfg.n_chunks + n) % 3]
                                       if last_blk else "ad"):
                         d_unit(qc, tt4, n, alt, pstag))
                        for tt4 in rng
                        for n in range(cfg.n_chunks)
                    ]
                if dunits:
                    units = interleave_even(units, dunits)
                if 1 <= b <= cfg.t_chunks:
                    qc = b - 1
                    merge(units, chain_b(qc), hpc * 2 * (qc + 1))
                else:
                    merge(units, None, 0)

    return nc


def rope_tables(T, dtype=np.float32):
    inv_freq = 1.0 / (ROPE_THETA ** (np.arange(0, D, 2, dtype=np.float32) / D))
    t = np.arange(T, dtype=np.float32)
    freqs = np.outer(t, inv_freq)  # [T, D/2]
    emb = np.concatenate([freqs, freqs], axis=-1)  # [T, D]
    return np.cos(emb).astype(dtype), np.sin(emb).astype(dtype)


def make_core_inputs(cfg: Cfg, x_b, w_qkv, w_out, cos, sin, hg):
    """Per-core input dict. x_b [T, C] fp32; w_qkv [C, 3C']; w_out [C', C];
    cos/sin [T, D]; hg = head-group index within the batch group."""
    T, C, hpc = cfg.T, cfg.C, cfg.hpc
    F = hpc * D
    H = w_qkv.shape[1] // 3 // D  # total heads in this (possibly shrunk) problem
    CQ = H * D

    f0 = hg * F
    t_chunks, c_tiles = T // 512, C // 128
    n_grp = c_tiles // 2
    xT = x_b.T.astype(BF_NP)  # [C, T]
    # xg: 2-cc groups, contiguous 2KB lines:
    #   xg[(tci*n_grp+g)*128 + p, e*512 + t'] = xT[(2g+e)*128 + p, tci*512+t']
    xg = np.ascontiguousarray(
        xT.reshape(n_grp, 2, 128, t_chunks, 512)
        .transpose(3, 0, 2, 1, 4)
        .reshape(t_chunks * n_grp * 128, 1024)
    )
    wq = w_qkv[:, f0 : f0 + F]
    wk = w_qkv[:, CQ + f0 : CQ + f0 + F]
    W = np.concatenate([wq, wk], axis=1)  # [C, 2F]
    # pack per-ft: wqkt[ft*128+p, cc*128+f] = W[cc*128+p, ft*128+f]
    nft, ncc = 2 * hpc, C // 128
    wqkt = np.ascontiguousarray(
        W.reshape(ncc, 128, nft, 128).transpose(2, 1, 0, 3).reshape(
            nft * 128, ncc * 128
        )
    ).astype(BF_NP)
    wv = w_qkv[:, 2 * CQ + f0 : 2 * CQ + f0 + F].astype(BF_NP)  # [C, F]
    # wvg[g*128+p, e*F+f] = wv[(2g+e)*128 + p, f]
    wvg = np.ascontiguousarray(
        wv.reshape(n_grp, 2, 128, F).transpose(0, 2, 1, 3).reshape(
            n_grp * 128, 2 * F
        )
    )
    wout = np.ascontiguousarray(w_out[f0 : f0 + F, :]).astype(BF_NP)

    cosT = cos.T.astype(BF_NP)  # [D, T]
    sinT = sin.T.astype(np.float32)
    sinT[0:64, :] *= -1.0  # bake rotate_half sign
    sinT = sinT.astype(BF_NP)
    # per-512-token-slice contiguous [t_chunks*128, 512]
    cosp = np.ascontiguousarray(
        cosT.reshape(128, t_chunks, 512).transpose(1, 0, 2).reshape(
            t_chunks * 128, 512
        )
    )
    sinp = np.ascontiguousarray(
        sinT.reshape(128, t_chunks, 512).transpose(1, 0, 2).reshape(
            t_chunks * 128, 512
        )
    )

    # triangle mask for the per-tile diagonal block: m[k, q'] = 1 iff k <= q'
    k_idx = np.arange(128)[:, None]
    q_idx = np.arange(128)[None, :]
    m = (k_idx <= q_idx).astype(BF_NP)

    return {
        "xg": xg,
        "wqkt": wqkt,
        "wvg": wvg,
        "wout": wout,
        "cosp": cosp,
        "sinp": sinp,
        "masks": np.ascontiguousarray(m),
    }


_NC_CACHE = {}


def _get_nc(cfg: Cfg):
    key = (cfg.T, cfg.C, cfg.hpc)
    if key not in _NC_CACHE:
        nc = build_attention(cfg)
        _split_multi_waits(nc)  # HW codegen needs ≤1 wait per instruction
        _NC_CACHE[key] = nc
    return _NC_CACHE[key]


def kernel(x, cos, sin, w_qkv, w_out, trace=False, tmpdir=None):
    """Full-problem entry point: full inputs in, full [B, T, C] output back."""
    from concourse.bass_utils import run_bass_kernel_spmd

    x = np.asarray(x, dtype=np.float32)
    cos = np.asarray(cos, dtype=np.float32)
    sin = np.asarray(sin, dtype=np.float32)
    w_qkv = np.asarray(w_qkv, dtype=np.float32)
    w_out = np.asarray(w_out, dtype=np.float32)

    cfg = Cfg()
    nc = _get_nc(cfg)

    in_maps = []
    for c in range(N_CORES):
        b, hg = c // 4, c % 4
        in_maps.append(
            make_core_inputs(cfg, x[b], w_qkv, w_out, cos, sin, hg)
        )

    res = run_bass_kernel_spmd(
        nc,
        in_maps,
        core_ids=list(range(N_CORES)),
        trace=trace,
        tmpdir=tmpdir,
    )
    partials = [np.asarray(r["out"], dtype=np.float32) for r in res.results]
    out = np.empty((B, cfg.T, cfg.C), dtype=np.float32)
    for b in range(B):
        out[b] = partials[4 * b] + partials[4 * b + 1]
        out[b] += partials[4 * b + 2]
        out[b] += partials[4 * b + 3]
    if trace:
        return out, res
    return out

